# revision 24
# baseline (speedup 1.0000x reference)
"""Trainium2 Bass kernel for AttentionDenseBlock (SE gate + offset conv + deform conv + tanh).

Strategy (per core, data-parallel over batch: 1 sample/core on 8 cores):
  - SE gate: spatial mean -> fc1 -> relu -> fc2 -> sigmoid; the per-channel
    scale s is FOLDED INTO THE WEIGHTS (wT, woffT scaled by s on ACT), so the
    input x is never rescaled and the DVE stays off the critical path.
  - Offset conv: 9 shifted bf16 matmuls accumulating in PSUM (s-scaled woffT).
  - Deform conv with |offset|<1 decomposes EXACTLY into a static 3x3-tap
    stencil per kernel position with data-dependent weights
    wy in {relu(-dy), 1-|dy|, relu(dy)} (x) wx likewise.
  - Main loop: for each kernel position k, the 9 modulated taps
    m_t = map_t (*) x_shift_t are accumulated in PSUM via IDENTITY matmuls
    (no weight reloads), the f32 sum z_k is copied to SBUF (ACT) and hit with
    ONE W_k matmul event: ~0.65us/tap on PE vs 1.31us/tap (full W matmul) or
    0.92us/tap (DVE adds). DVE does only the 9 modulate multiplies per k,
    merged into 4 wide multi-tap ops (4 free dims).
  - Spatial processed in 8 "eighths" of 7 rows (HN=392 = one PSUM bank f32):
    out-psum (4 banks) + z-psum (2) + offset-psum (2) = all 8 banks.
  - PSUM->SBUF z copies hidden under the next W matmul group via the
    interleave: id(k,j0) W(k-1,j1) id(k,j1) W(k,j0).
  - Tap maps are built per quarter on 9-partition tiles (ACT relus + one
    9-tap DVE product op), stored to DRAM in one DMA, and broadcast to 128
    partitions with one contiguous-read DMA per (q,k) on 3 rotating queues.
"""

import os
import sys
from collections import deque
from contextlib import ExitStack

import numpy as np

sys.path.insert(0, "/opt/trn_rl_repo")

import concourse.bass as bass
import concourse.bacc as bacc
import concourse.mybir as mybir
import concourse.tile as tile
from concourse.masks import make_identity

B, C, O, H, W = 8, 256, 256, 56, 56
KH = KW = 3
K2 = 9
HP, WP = H + 4, W + 4  # zero-padded by 2 for the 5x5 shift range
HW = H * W
QROWS = 14            # rows per quarter (map granularity)
QN = HW // 4          # 784 spatial positions per quarter
HROWS = 7             # rows per eighth (psum granularity)
HN = QN // 2          # 392 = one PSUM bank of f32
RED = 16              # SE reduction dim

F32 = mybir.dt.float32
BF16 = mybir.dt.bfloat16
AF = mybir.ActivationFunctionType
ALU = mybir.AluOpType

LAST_RESULT = None


def _ap(base, extra_dims):
    """AP on `base`'s tensor starting at base's offset with explicit free dims."""
    return bass.AP(tensor=base.tensor, offset=base.offset,
                   ap=[list(base.ap[0])] + [list(d) for d in extra_dims])


def build():
    nc = bacc.Bacc()
    x_d = nc.dram_tensor("x", (1, C, H, W), F32, kind="ExternalInput")
    woff_d = nc.dram_tensor("w_off", (2 * K2, C, KH, KW), F32, kind="ExternalInput")
    boff_d = nc.dram_tensor("b_off", (2 * K2,), F32, kind="ExternalInput")
    wconv_d = nc.dram_tensor("w_conv", (O, C, KH, KW), F32, kind="ExternalInput")
    bconv_d = nc.dram_tensor("b_conv", (O,), F32, kind="ExternalInput")
    fc1_d = nc.dram_tensor("fc1", (RED, C), F32, kind="ExternalInput")
    fc2_d = nc.dram_tensor("fc2", (C, RED), F32, kind="ExternalInput")
    out_d = nc.dram_tensor("out", (1, O, H, W), F32, kind="ExternalOutput")

    with tile.TileContext(nc) as tc, ExitStack() as ctx:
        singles = ctx.enter_context(tc.tile_pool(name="singles", bufs=1))
        wyxpool = ctx.enter_context(tc.tile_pool(name="wyxpool", bufs=1))
        mpool = ctx.enter_context(tc.tile_pool(name="mpool", bufs=2))
        reppool = ctx.enter_context(tc.tile_pool(name="reppool", bufs=6))
        zsbpool = ctx.enter_context(tc.tile_pool(name="zsbpool", bufs=3))
        outpool = ctx.enter_context(tc.tile_pool(name="outpool", bufs=2))
        dpool = ctx.enter_context(tc.tile_pool(name="dpool", bufs=1, space="DRAM"))
        psum_z = ctx.enter_context(tc.tile_pool(name="psum_z", bufs=1, space="PSUM"))
        psum_out = ctx.enter_context(tc.tile_pool(name="psum_out", bufs=1, space="PSUM"))
        psum_off = ctx.enter_context(tc.tile_pool(name="psum_off", bufs=2, space="PSUM"))

        # ---- static tiles ----
        xs_bf = singles.tile([128, 2, HP, WP], BF16)     # raw x, padded, even phase
        xs_bf1 = singles.tile([128, 2, HP, WP - 2], BF16)  # odd phase: col c = col c+1
        wT = singles.tile([128, 2, K2, O], BF16)         # [c, cc, k, o]; s-scaled
        # woffT free dim: [0:9]=dy weights, [32:41]=dx weights (aligned blocks)
        woffT = singles.tile([128, 2, K2, 48], BF16)
        fc1T = singles.tile([128, 2, RED], F32)
        fc2T = singles.tile([128, C], F32)
        bconv = singles.tile([128, 2], F32)
        boff = singles.tile([64, 1], F32)
        y_se = singles.tile([128, 2, 1], F32)
        h_se = singles.tile([128, 1], F32)
        s_se = singles.tile([128, 2, 1], F32)
        boffn = singles.tile([64, 1], F32)
        ident = singles.tile([128, 128], BF16)
        wnat2 = mpool.tile([128, 2, C * K2], BF16, name="wnat2", tag="wn",
                           bufs=1)
        woff_nat2 = mpool.tile([2 * K2, C * K2], BF16, name="woff_nat2", tag="m",
                               padded_shape=[2 * K2, 2 * K2 * QN])
        # k-major tap maps per quarter: row k*9 + (3r+s)
        maps_dram = [dpool.tile([K2 * K2, QN], BF16, name=f"maps{q}") for q in range(4)]

        make_identity(nc, ident[:, :])

        # ---- weight DMA first (cast to bf16 on SWDGE) so PE transposes start
        # early, then the big x input DMA ----
        wc_r = wconv_d[:].rearrange("o c kh kw -> o (c kh kw)")
        nc.gpsimd.dma_start(out=woff_nat2[:, :],
                            in_=woff_d[:].rearrange("o c kh kw -> o (c kh kw)"))
        for oc in range(2):
            nc.gpsimd.dma_start(out=wnat2[:, oc, :], in_=wc_r[oc * 128:(oc + 1) * 128, :])
        nc.vector.memset(xs_bf[:, :, :, :], 0.0)
        nc.vector.memset(xs_bf1[:, :, :, :], 0.0)
        x_r = x_d[:].rearrange("one c h w -> (one c) h w")
        HH = H // 2
        for hh in range(2):
            nc.gpsimd.dma_start(
                out=xs_bf[:, 0, 2 + hh * HH:2 + (hh + 1) * HH, 2:2 + W],
                in_=x_r[0:128, hh * HH:(hh + 1) * HH, :])
        xstage = reppool.tile([128, H, W], F32, name="xstage", tag="rep")
        for hh in range(2):
            nc.sync.dma_start(out=xstage[:, hh * HH:(hh + 1) * HH, :],
                              in_=x_r[128:256, hh * HH:(hh + 1) * HH, :])
            nc.scalar.activation(
                xs_bf[:, 1, 2 + hh * HH:2 + (hh + 1) * HH, 2:2 + W],
                xstage[:, hh * HH:(hh + 1) * HH, :], AF.Copy)
        fc1_r = fc1_d[:].rearrange("m c -> c m")
        for cc in range(2):
            nc.sync.dma_start(out=fc1T[:, cc, :], in_=fc1_r[cc * 128:(cc + 1) * 128, :])
        nc.vector.memset(fc2T[:, :], 0.0)
        nc.sync.dma_start(out=fc2T[0:RED, :], in_=fc2_d[:].rearrange("c m -> m c"))
        nc.sync.dma_start(out=bconv[:, :],
                          in_=bconv_d[:].rearrange("(a c) -> c a", a=2))
        # b_off loaded de-interleaved: dy biases -> rows 0:9, dx -> rows 32:41
        nc.vector.memset(boff[:, :], 0.0)
        boff_src = boff_d[:]
        nc.sync.dma_start(out=boff[0:K2, 0:1],
                          in_=bass.AP(tensor=boff_src.tensor, offset=boff_src.offset,
                                      ap=[[2, K2], [0, 1]]))
        nc.sync.dma_start(out=boff[32:32 + K2, 0:1],
                          in_=bass.AP(tensor=boff_src.tensor,
                                      offset=boff_src.offset + 1,
                                      ap=[[2, K2], [0, 1]]))

        nc.vector.memset(woffT[:, :, :, :], 0.0)

        # ---- offset-weight transposes (gate the offset conv), batched ----
        for cc in range(2):
            srco = woff_nat2[:, :].rearrange("p (c k) -> p c k", k=K2)
            for g in range(3):
                tpo = psum_off.tile([128, 3, 128], BF16, tag="off", name="tpo")
                for i in range(3):
                    kk = 3 * g + i
                    nc.tensor.transpose(tpo[:, i, 0:2 * K2],
                                        srco[:, cc * 128:(cc + 1) * 128, kk],
                                        ident[0:2 * K2, 0:2 * K2])
                # de-interleave offset channels: dy -> cols 0:9, dx -> 32:41
                nc.vector.tensor_copy(
                    _ap(woffT[:, cc, 3 * g, 0], [[48, 3], [1, K2]]),
                    _ap(tpo[:, 0, 0], [[128, 3], [2, K2]]))
                nc.vector.tensor_copy(
                    _ap(woffT[:, cc, 3 * g, 32], [[48, 3], [1, K2]]),
                    _ap(tpo[:, 0, 1], [[128, 3], [2, K2]]))

        # odd column phase of raw x (cols 0..57 used; col 58/59 never read)
        for cc in range(2):
            nc.scalar.activation(xs_bf1[:, cc, :, 0:WP - 2], xs_bf[:, cc, :, 1:WP - 1],
                                 AF.Copy)

        # ---- SE gate -> s; fold s into wTs / woffTs on ACT ----
        y_part = singles.tile([128, 2, 2], F32)
        for cc in range(2):
            for hh in range(2):
                nc.vector.tensor_reduce(
                    out=y_part[:, cc, hh:hh + 1],
                    in_=xs_bf[:, cc, 2 + hh * (H // 2):2 + (hh + 1) * (H // 2), 2:2 + W],
                    axis=mybir.AxisListType.XY, op=ALU.add)
        for cc in range(2):
            nc.vector.tensor_reduce(out=y_se[:, cc, 0:1], in_=y_part[:, cc, :],
                                    axis=mybir.AxisListType.X, op=ALU.add)
        nc.vector.tensor_scalar_mul(y_se[:, :, 0:1], y_se[:, :, 0:1], 1.0 / HW)
        h_ps = psum_z.tile([128, RED], F32, tag="z")
        for cc in range(2):
            nc.tensor.matmul(h_ps[0:RED, 0:1], lhsT=fc1T[:, cc, :], rhs=y_se[:, cc, 0:1],
                             start=(cc == 0), stop=(cc == 1))
        nc.vector.memset(h_se[:, :], 0.0)
        nc.vector.tensor_relu(h_se[0:RED, 0:1], h_ps[0:RED, 0:1])
        for cc in range(2):
            s_ps = psum_z.tile([128, RED], F32, tag="z")
            nc.tensor.matmul(s_ps[:, 0:1], lhsT=fc2T[:, cc * 128:(cc + 1) * 128],
                             rhs=h_se[:, 0:1], start=True, stop=True)
            nc.scalar.activation(s_se[:, cc, 0:1], s_ps[:, 0:1], AF.Sigmoid)
        for cc in range(2):
            nc.scalar.activation(woffT[:, cc, :, 0:48], woffT[:, cc, :, 0:48],
                                 AF.Copy, scale=s_se[:, cc, 0:1])

        nc.scalar.activation(boffn[:, 0:1], boff[:, 0:1], AF.Copy, scale=-1.0)

        # ---- per-quarter preamble, staged: offset conv -> maps -> DRAM ----
        def preamble_ab(q, nn, wyf, wxf):
            if True:
                off_ps = psum_off.tile([64, HN], F32, tag="off")
                for kk in range(K2):
                    ki, kj = divmod(kk, 3)
                    dh, dw = ki - 1, kj - 1
                    for cc in range(2):
                        r0 = 2 + dh + q * QROWS + nn * HROWS
                        rhs = xs_bf[:, cc, r0:r0 + HROWS, 2 + dw:2 + dw + W]
                        nc.tensor.matmul(off_ps[0:48, :],
                                         lhsT=woffT[:, cc, kk, 0:48], rhs=rhs,
                                         start=(kk == 0 and cc == 0),
                                         stop=(kk == K2 - 1 and cc == 1))
                # offset = psum + b_off, fused into relu(+-offset) tap weights
                nsl = slice(nn * HN, (nn + 1) * HN)
                nc.scalar.activation(wyf[:, 0, nsl], off_ps[0:K2, :], AF.Relu,
                                     scale=-1.0, bias=boffn[0:K2, 0:1])
                nc.scalar.activation(wyf[:, 2, nsl], off_ps[0:K2, :], AF.Relu,
                                     scale=1.0, bias=boff[0:K2, 0:1])
                nc.scalar.activation(wxf[:, 0, nsl], off_ps[32:32 + K2, :], AF.Relu,
                                     scale=-1.0, bias=boffn[32:32 + K2, 0:1])
                nc.scalar.activation(wxf[:, 2, nsl], off_ps[32:32 + K2, :], AF.Relu,
                                     scale=1.0, bias=boff[32:32 + K2, 0:1])
        def preamble_c(q, wyf, wxf):
            # wy1 = 1 - |dy| = 1 - (relu(dy) + relu(-dy)); same for wx1
            nc.gpsimd.tensor_add(wyf[:, 1, :], wyf[:, 0, :], wyf[:, 2, :])
            nc.scalar.activation(wyf[:, 1, :], wyf[:, 1, :], AF.Copy, scale=-1.0,
                                 bias=1.0)
            nc.gpsimd.tensor_add(wxf[:, 1, :], wxf[:, 0, :], wxf[:, 2, :])
            nc.scalar.activation(wxf[:, 1, :], wxf[:, 1, :], AF.Copy, scale=-1.0,
                                 bias=1.0)
            # all 9 tap products in ONE DVE op (sigma-major rows t = 3s + r),
            # one DMA store k-major
            mprod = mpool.tile([K2, K2, QN], BF16, name="mprod", tag="m")
            win = _ap(wyf[0:K2, 0, 0], [[0, 3], [QN, 3], [1, QN]])
            xin = _ap(wxf[0:K2, 0, 0], [[QN, 3], [0, 3], [1, QN]])
            mout = _ap(mprod[0:K2, 0, 0], [[3 * QN, 3], [QN, 3], [1, QN]])
            nc.gpsimd.tensor_tensor(mout, win, xin, op=ALU.mult)
            nc.scalar.dma_start(out=maps_dram[q][:], in_=mprod[:, :, :])

        # ---- map broadcast: 3 sigma-group sub-DMAs on one queue, so the
        # first modulate op only waits on the first third ----
        def bcast(q, k, engine):
            md = maps_dram[q][0:1, 0:1]
            rep = reppool.tile([128, K2, QN], BF16, name="rep")
            flat = rep[:, :, :].rearrange("p a b -> p (a b)")
            for sg in range(3):
                engine.dma_start(
                    out=flat[:, 3 * sg * QN:3 * (sg + 1) * QN],
                    in_=bass.AP(tensor=md.tensor,
                                offset=md.offset + (k * K2 + 3 * sg) * QN,
                                ap=[[0, 128], [1, 3 * QN]]))
            return rep

        # ---- modulate: m[k][cc, t, :] = map_t (*) x window, 4 wide ops/k ----
        def mults(q, k, rep):
            ki, kj = divmod(k, 3)
            m = mpool.tile([128, 2, K2, QN], BF16, name="m", tag="m")
            r0 = q * QROWS + ki          # padded row of tap (rho=0, h=0)
            for cc in range(2):
                for sg in range(3):      # 3-tap (rho) column group per op
                    cs = kj + sg
                    xt, cb, rst = ((xs_bf, cs, WP) if cs % 2 == 0
                                   else (xs_bf1, cs - 1, WP - 2))
                    xin = _ap(xt[:, cc, r0, cb],
                              [[rst, 3], [rst, QROWS], [1, W]])
                    min_ = _ap(rep[:, 3 * sg, 0],
                               [[QN, 3], [W, QROWS], [1, W]])
                    mout = _ap(m[:, cc, 3 * sg, 0],
                               [[QN, 3], [W, QROWS], [1, W]])
                    nc.vector.tensor_tensor(mout, xin, min_, op=ALU.mult)
            return m

        # ---- main eighth bodies ----
        out_r = out_d[:].rearrange("one o h w -> (one o) h w")

        def id_group(m, k, j):
            """9 identity matmuls per cc accumulating taps into z psum."""
            zps = psum_z.tile([128, 2, HN], F32, tag="z", padded_shape=[128, 2, 512])
            for cc in range(2):
                for t in range(K2):
                    nc.tensor.matmul(zps[:, cc, :], lhsT=ident[:, :],
                                     rhs=m[:, cc, t, j * HN:(j + 1) * HN],
                                     start=(t == 0), stop=(t == K2 - 1))
            z_sb = zsbpool.tile([128, 2, HN], BF16, name="z_sb")
            nc.scalar.activation(z_sb[:, :, :], zps[:, :, :], AF.Copy)
            return z_sb

        out_ps = {}

        def w_group(q, k, j, z_sb):
            if k == 0:
                for oc in range(2):
                    out_ps[(j, oc)] = psum_out.tile([128, HN], F32, tag=f"o{oc}j{j}",
                                                    name=f"ops{oc}{j}")
            for cc in range(2):
                for oc in range(2):
                    nc.tensor.matmul(out_ps[(j, oc)][:, :],
                                     lhsT=wT[:, cc, k, oc * 128:(oc + 1) * 128],
                                     rhs=z_sb[:, cc, :],
                                     start=(k == 0 and cc == 0),
                                     stop=(k == K2 - 1 and cc == 1))
            if k == K2 - 1:
                e = 2 * q + j
                for oc in range(2):
                    osb = outpool.tile([128, HN], F32, name="osb")
                    nc.scalar.activation(osb[:, :], out_ps[(j, oc)][:, :], AF.Tanh,
                                         bias=bconv[:, oc:oc + 1])
                    nc.sync.dma_start(
                        out=out_r[oc * 128:(oc + 1) * 128,
                                  e * HROWS:(e + 1) * HROWS, :],
                        in_=osb[:, :])

        # ---- software-pipelined emission ----
        bcast_engines = [nc.gpsimd, nc.sync]
        bq = deque((q, k) for q in range(4) for k in range(K2))
        reps = {}
        n_bcast = 0

        def issue_bcast(ready_q):
            nonlocal n_bcast
            if bq and bq[0][0] <= ready_q:
                q, k = bq.popleft()
                reps[(q, k)] = bcast(q, k, bcast_engines[n_bcast % 2])
                n_bcast += 1

        def preamble(q):
            wyf = wyxpool.tile([K2, 3, QN], BF16, tag="wyf")
            wxf = wyxpool.tile([K2, 3, QN], BF16, tag="wxf")
            preamble_ab(q, 0, wyf, wxf)
            preamble_ab(q, 1, wyf, wxf)
            preamble_c(q, wyf, wxf)

        preamble(0)

        # ---- conv-weight transposes (needed only by the first W group) ----
        for cc in range(2):
            for g in range(5):
                kks = [2 * g, 2 * g + 1] if g < 4 else [8]
                tpw = psum_off.tile([128, 2 * len(kks), 128], BF16, tag="off",
                                    name="tpw")
                si = 0
                for kk in kks:
                    for oc in range(2):
                        srcw = wnat2[:, oc, :].rearrange("p (c k) -> p c k", k=K2)
                        nc.tensor.transpose(tpw[:, si, :],
                                            srcw[:, cc * 128:(cc + 1) * 128, kk],
                                            ident[:, :])
                        si += 1
                nc.vector.tensor_copy(
                    _ap(wT[:, cc, kks[0], 0], [[1, si * 128]]),
                    _ap(tpw[:, 0, 0], [[1, si * 128]]))
        for cc in range(2):
            nc.scalar.activation(wT[:, cc, :, :], wT[:, cc, :, :], AF.Copy,
                                 scale=s_se[:, cc, 0:1])

        ready = 0                     # highest q whose maps are stored
        pre_tiles = None
        for _ in range(5):
            issue_bcast(ready)
        ms = {}
        ms[(0, 0)] = mults(0, 0, reps.pop((0, 0)))
        pending_w = None              # (q, k, j, z_sb) awaiting emission

        for q in range(4):
            for k in range(K2):
                # lookahead: keep broadcast pipeline full, mults k+1
                issue_bcast(ready)
                issue_bcast(ready)
                nq, nk = (q, k + 1) if k + 1 < K2 else (q + 1, 0)
                if nq < 4:
                    ms[(nq, nk)] = mults(nq, nk, reps.pop((nq, nk)))
                m = ms.pop((q, k))
                z0 = id_group(m, k, 0)
                if pending_w is not None:
                    w_group(*pending_w)
                z1 = id_group(m, k, 1)
                w_group(q, k, 0, z0)
                pending_w = (q, k, 1, z1)
                if q < 3:
                    if k == 1:
                        pre_tiles = (wyxpool.tile([K2, 3, QN], BF16, tag="wyf",
                                                  name="wyf"),
                                     wyxpool.tile([K2, 3, QN], BF16, tag="wxf",
                                                  name="wxf"))
                        preamble_ab(q + 1, 0, *pre_tiles)
                    elif k == 2:
                        preamble_ab(q + 1, 1, *pre_tiles)
                    elif k == 3:
                        preamble_c(q + 1, *pre_tiles)
                        ready = q + 1
        w_group(*pending_w)
    nc.finalize()
    return nc


_NC = None


def _get_nc():
    global _NC
    if _NC is None:
        _NC = build()
    return _NC


def kernel(**inputs):
    global LAST_RESULT
    from concourse.bass_utils import run_bass_kernel_spmd

    nc = _get_nc()
    x = np.ascontiguousarray(inputs["x"], dtype=np.float32)
    shared = {k: np.ascontiguousarray(np.asarray(inputs[k]), dtype=np.float32)
              for k in ("w_off", "b_off", "w_conv", "b_conv", "fc1", "fc2")}
    in_maps = [{"x": x[i:i + 1], **shared} for i in range(B)]
    res = run_bass_kernel_spmd(nc, in_maps, core_ids=list(range(B)),
                               trace=bool(int(os.environ.get("KB_TRACE", "0"))))
    LAST_RESULT = res
    out = np.concatenate([res.results[i]["out"] for i in range(B)], axis=0)
    return out.astype(np.float32)


if __name__ == "__main__":
    nc = build()
    print("build OK")


# revision 25
# speedup vs baseline: 1.0623x; 1.0623x over previous
"""Trainium2 Bass kernel for AttentionDenseBlock (SE gate + offset conv + deform conv + tanh).

Strategy (per core, data-parallel over batch: 1 sample/core on 8 cores):
  - SE gate: spatial mean -> fc1 -> relu -> fc2 -> sigmoid; the per-channel
    scale s is FOLDED INTO THE WEIGHTS (wT, woffT scaled by s on ACT), so the
    input x is never rescaled and the DVE stays off the critical path.
  - Offset conv: 9 shifted bf16 matmuls accumulating in PSUM (s-scaled woffT).
  - Deform conv with |offset|<1 decomposes EXACTLY into a static 3x3-tap
    stencil per kernel position with data-dependent weights
    wy in {relu(-dy), 1-|dy|, relu(dy)} (x) wx likewise.
  - Main loop: for each kernel position k, the 9 modulated taps
    m_t = map_t (*) x_shift_t are accumulated in PSUM via IDENTITY matmuls
    (no weight reloads), the f32 sum z_k is copied to SBUF (ACT) and hit with
    ONE W_k matmul event: ~0.65us/tap on PE vs 1.31us/tap (full W matmul) or
    0.92us/tap (DVE adds). DVE does only the 9 modulate multiplies per k,
    merged into 4 wide multi-tap ops (4 free dims).
  - Spatial processed in 8 "eighths" of 7 rows (HN=392 = one PSUM bank f32):
    out-psum (4 banks) + z-psum (2) + offset-psum (2) = all 8 banks.
  - PSUM->SBUF z copies hidden under the next W matmul group via the
    interleave: id(k,j0) W(k-1,j1) id(k,j1) W(k,j0).
  - Tap maps are built per quarter on 9-partition tiles (ACT relus + one
    9-tap DVE product op), stored to DRAM in one DMA, and broadcast to 128
    partitions with one contiguous-read DMA per (q,k) on 3 rotating queues.
"""

import os
import sys
from collections import deque
from contextlib import ExitStack

import numpy as np

sys.path.insert(0, "/opt/trn_rl_repo")

import concourse.bass as bass
import concourse.bacc as bacc
import concourse.mybir as mybir
import concourse.tile as tile
from concourse.masks import make_identity

B, C, O, H, W = 8, 256, 256, 56, 56
KH = KW = 3
K2 = 9
HP, WP = H + 4, W + 4  # zero-padded by 2 for the 5x5 shift range
HW = H * W
QROWS = 14            # rows per quarter (map granularity)
QN = HW // 4          # 784 spatial positions per quarter
HROWS = 7             # rows per eighth (psum granularity)
HN = QN // 2          # 392 = one PSUM bank of f32
RED = 16              # SE reduction dim

F32 = mybir.dt.float32
BF16 = mybir.dt.bfloat16
AF = mybir.ActivationFunctionType
ALU = mybir.AluOpType

LAST_RESULT = None


def _ap(base, extra_dims):
    """AP on `base`'s tensor starting at base's offset with explicit free dims."""
    return bass.AP(tensor=base.tensor, offset=base.offset,
                   ap=[list(base.ap[0])] + [list(d) for d in extra_dims])


def build():
    nc = bacc.Bacc()
    x_d = nc.dram_tensor("x", (1, C, H, W), F32, kind="ExternalInput")
    woff_d = nc.dram_tensor("w_off", (2 * K2, C, KH, KW), F32, kind="ExternalInput")
    boff_d = nc.dram_tensor("b_off", (2 * K2,), F32, kind="ExternalInput")
    wconv_d = nc.dram_tensor("w_conv", (O, C, KH, KW), F32, kind="ExternalInput")
    bconv_d = nc.dram_tensor("b_conv", (O,), F32, kind="ExternalInput")
    fc1_d = nc.dram_tensor("fc1", (RED, C), F32, kind="ExternalInput")
    fc2_d = nc.dram_tensor("fc2", (C, RED), F32, kind="ExternalInput")
    out_d = nc.dram_tensor("out", (1, O, H, W), F32, kind="ExternalOutput")

    with tile.TileContext(nc) as tc, ExitStack() as ctx:
        singles = ctx.enter_context(tc.tile_pool(name="singles", bufs=1))
        wyxpool = ctx.enter_context(tc.tile_pool(name="wyxpool", bufs=1))
        mpool = ctx.enter_context(tc.tile_pool(name="mpool", bufs=2))
        reppool = ctx.enter_context(tc.tile_pool(name="reppool", bufs=6))
        zsbpool = ctx.enter_context(tc.tile_pool(name="zsbpool", bufs=3))
        outpool = ctx.enter_context(tc.tile_pool(name="outpool", bufs=2))
        dpool = ctx.enter_context(tc.tile_pool(name="dpool", bufs=1, space="DRAM"))
        psum_z = ctx.enter_context(tc.tile_pool(name="psum_z", bufs=1, space="PSUM"))
        psum_out = ctx.enter_context(tc.tile_pool(name="psum_out", bufs=1, space="PSUM"))
        psum_off = ctx.enter_context(tc.tile_pool(name="psum_off", bufs=2, space="PSUM"))

        # ---- static tiles ----
        xs_bf = singles.tile([128, 2, HP, WP], BF16)     # raw x, padded, even phase
        xs_bf1 = singles.tile([128, 2, HP, WP - 2], BF16)  # odd phase: col c = col c+1
        wT = singles.tile([128, 2, K2, O], BF16)         # [c, cc, k, o]; s-scaled
        # woffT free dim: [0:9]=dy weights, [32:41]=dx weights (aligned blocks)
        woffT = singles.tile([128, 2, K2, 48], BF16)
        fc1T = singles.tile([128, 2, RED], F32)
        fc2T = singles.tile([128, C], F32)
        bconv = singles.tile([128, 2], F32)
        boff = singles.tile([64, 1], F32)
        y_se = singles.tile([128, 2, 1], F32)
        h_se = singles.tile([128, 1], F32)
        s_se = singles.tile([128, 2, 1], F32)
        boffn = singles.tile([64, 1], F32)
        ident = singles.tile([128, 128], BF16)
        wnat2 = mpool.tile([128, 2, C * K2], BF16, name="wnat2", tag="wn",
                           bufs=1)
        woff_nat2 = mpool.tile([2 * K2, C * K2], BF16, name="woff_nat2", tag="m",
                               padded_shape=[2 * K2, 2 * K2 * QN])
        # k-major tap maps per quarter: row k*9 + (3r+s)
        maps_dram = [dpool.tile([K2 * K2, QN], BF16, name=f"maps{q}") for q in range(4)]

        make_identity(nc, ident[:, :])

        # ---- weight DMA first (cast to bf16 on SWDGE) so PE transposes start
        # early, then the big x input DMA ----
        wc_r = wconv_d[:].rearrange("o c kh kw -> o (c kh kw)")
        nc.gpsimd.dma_start(out=woff_nat2[:, :],
                            in_=woff_d[:].rearrange("o c kh kw -> o (c kh kw)"))
        for oc in range(2):
            nc.gpsimd.dma_start(out=wnat2[:, oc, :], in_=wc_r[oc * 128:(oc + 1) * 128, :])
        nc.vector.memset(xs_bf[:, :, :, :], 0.0)
        nc.vector.memset(xs_bf1[:, :, :, :], 0.0)
        x_r = x_d[:].rearrange("one c h w -> (one c) h w")
        HH = H // 2
        for hh in range(2):
            nc.gpsimd.dma_start(
                out=xs_bf[:, 0, 2 + hh * HH:2 + (hh + 1) * HH, 2:2 + W],
                in_=x_r[0:128, hh * HH:(hh + 1) * HH, :])
        xstage = reppool.tile([128, H, W], F32, name="xstage", tag="rep")
        for hh in range(2):
            nc.sync.dma_start(out=xstage[:, hh * HH:(hh + 1) * HH, :],
                              in_=x_r[128:256, hh * HH:(hh + 1) * HH, :])
            nc.scalar.activation(
                xs_bf[:, 1, 2 + hh * HH:2 + (hh + 1) * HH, 2:2 + W],
                xstage[:, hh * HH:(hh + 1) * HH, :], AF.Copy)
        fc1_r = fc1_d[:].rearrange("m c -> c m")
        for cc in range(2):
            nc.sync.dma_start(out=fc1T[:, cc, :], in_=fc1_r[cc * 128:(cc + 1) * 128, :])
        nc.vector.memset(fc2T[:, :], 0.0)
        nc.sync.dma_start(out=fc2T[0:RED, :], in_=fc2_d[:].rearrange("c m -> m c"))
        nc.sync.dma_start(out=bconv[:, :],
                          in_=bconv_d[:].rearrange("(a c) -> c a", a=2))
        # b_off loaded de-interleaved: dy biases -> rows 0:9, dx -> rows 32:41
        nc.vector.memset(boff[:, :], 0.0)
        boff_src = boff_d[:]
        nc.sync.dma_start(out=boff[0:K2, 0:1],
                          in_=bass.AP(tensor=boff_src.tensor, offset=boff_src.offset,
                                      ap=[[2, K2], [0, 1]]))
        nc.sync.dma_start(out=boff[32:32 + K2, 0:1],
                          in_=bass.AP(tensor=boff_src.tensor,
                                      offset=boff_src.offset + 1,
                                      ap=[[2, K2], [0, 1]]))

        nc.vector.memset(woffT[:, :, :, :], 0.0)

        # ---- offset-weight transposes (gate the offset conv), batched ----
        for cc in range(2):
            srco = woff_nat2[:, :].rearrange("p (c k) -> p c k", k=K2)
            for g in range(3):
                tpo = psum_off.tile([128, 3, 128], BF16, tag="off", name="tpo")
                for i in range(3):
                    kk = 3 * g + i
                    nc.tensor.transpose(tpo[:, i, 0:2 * K2],
                                        srco[:, cc * 128:(cc + 1) * 128, kk],
                                        ident[0:2 * K2, 0:2 * K2])
                # de-interleave offset channels: dy -> cols 0:9, dx -> 32:41
                nc.vector.tensor_copy(
                    _ap(woffT[:, cc, 3 * g, 0], [[48, 3], [1, K2]]),
                    _ap(tpo[:, 0, 0], [[128, 3], [2, K2]]))
                nc.vector.tensor_copy(
                    _ap(woffT[:, cc, 3 * g, 32], [[48, 3], [1, K2]]),
                    _ap(tpo[:, 0, 1], [[128, 3], [2, K2]]))

        # odd column phase of raw x (cols 0..57 used; col 58/59 never read)
        for cc in range(2):
            nc.scalar.activation(xs_bf1[:, cc, :, 0:WP - 2], xs_bf[:, cc, :, 1:WP - 1],
                                 AF.Copy)

        # ---- SE gate -> s; fold s into wTs / woffTs on ACT ----
        y_part = singles.tile([128, 2, 2], F32)
        for cc in range(2):
            for hh in range(2):
                nc.vector.tensor_reduce(
                    out=y_part[:, cc, hh:hh + 1],
                    in_=xs_bf[:, cc, 2 + hh * (H // 2):2 + (hh + 1) * (H // 2), 2:2 + W],
                    axis=mybir.AxisListType.XY, op=ALU.add)
        for cc in range(2):
            nc.vector.tensor_reduce(out=y_se[:, cc, 0:1], in_=y_part[:, cc, :],
                                    axis=mybir.AxisListType.X, op=ALU.add)
        nc.vector.tensor_scalar_mul(y_se[:, :, 0:1], y_se[:, :, 0:1], 1.0 / HW)
        h_ps = psum_z.tile([128, RED], F32, tag="z")
        for cc in range(2):
            nc.tensor.matmul(h_ps[0:RED, 0:1], lhsT=fc1T[:, cc, :], rhs=y_se[:, cc, 0:1],
                             start=(cc == 0), stop=(cc == 1))
        nc.vector.memset(h_se[:, :], 0.0)
        nc.vector.tensor_relu(h_se[0:RED, 0:1], h_ps[0:RED, 0:1])
        for cc in range(2):
            s_ps = psum_z.tile([128, RED], F32, tag="z")
            nc.tensor.matmul(s_ps[:, 0:1], lhsT=fc2T[:, cc * 128:(cc + 1) * 128],
                             rhs=h_se[:, 0:1], start=True, stop=True)
            nc.scalar.activation(s_se[:, cc, 0:1], s_ps[:, 0:1], AF.Sigmoid)
        for cc in range(2):
            nc.scalar.activation(woffT[:, cc, :, 0:48], woffT[:, cc, :, 0:48],
                                 AF.Copy, scale=s_se[:, cc, 0:1])

        nc.scalar.activation(boffn[:, 0:1], boff[:, 0:1], AF.Copy, scale=-1.0)

        # ---- per-quarter preamble, staged: offset conv -> maps -> DRAM ----
        def preamble_ab(q, nn, wyf, wxf):
            if True:
                off_ps = psum_off.tile([64, HN], F32, tag="off")
                for kk in range(K2):
                    ki, kj = divmod(kk, 3)
                    dh, dw = ki - 1, kj - 1
                    for cc in range(2):
                        r0 = 2 + dh + q * QROWS + nn * HROWS
                        rhs = xs_bf[:, cc, r0:r0 + HROWS, 2 + dw:2 + dw + W]
                        nc.tensor.matmul(off_ps[0:48, :],
                                         lhsT=woffT[:, cc, kk, 0:48], rhs=rhs,
                                         start=(kk == 0 and cc == 0),
                                         stop=(kk == K2 - 1 and cc == 1))
                # offset = psum + b_off, fused into relu(+-offset) tap weights
                nsl = slice(nn * HN, (nn + 1) * HN)
                nc.scalar.activation(wyf[:, 0, nsl], off_ps[0:K2, :], AF.Relu,
                                     scale=-1.0, bias=boffn[0:K2, 0:1])
                nc.scalar.activation(wyf[:, 2, nsl], off_ps[0:K2, :], AF.Relu,
                                     scale=1.0, bias=boff[0:K2, 0:1])
                nc.scalar.activation(wxf[:, 0, nsl], off_ps[32:32 + K2, :], AF.Relu,
                                     scale=-1.0, bias=boffn[32:32 + K2, 0:1])
                nc.scalar.activation(wxf[:, 2, nsl], off_ps[32:32 + K2, :], AF.Relu,
                                     scale=1.0, bias=boff[32:32 + K2, 0:1])
        def preamble_c(q, wyf, wxf):
            # wy1 = 1 - |dy| = 1 - (relu(dy) + relu(-dy)); same for wx1
            nc.vector.tensor_add(wyf[:, 1, :], wyf[:, 0, :], wyf[:, 2, :])
            nc.scalar.activation(wyf[:, 1, :], wyf[:, 1, :], AF.Copy, scale=-1.0,
                                 bias=1.0)
            nc.vector.tensor_add(wxf[:, 1, :], wxf[:, 0, :], wxf[:, 2, :])
            nc.scalar.activation(wxf[:, 1, :], wxf[:, 1, :], AF.Copy, scale=-1.0,
                                 bias=1.0)
            # all 9 tap products in ONE DVE op (sigma-major rows t = 3s + r),
            # one DMA store k-major
            mprod = mpool.tile([K2, K2, QN], BF16, name="mprod", tag="m")
            win = _ap(wyf[0:K2, 0, 0], [[0, 3], [QN, 3], [1, QN]])
            xin = _ap(wxf[0:K2, 0, 0], [[QN, 3], [0, 3], [1, QN]])
            mout = _ap(mprod[0:K2, 0, 0], [[3 * QN, 3], [QN, 3], [1, QN]])
            nc.vector.tensor_tensor(mout, win, xin, op=ALU.mult)
            nc.scalar.dma_start(out=maps_dram[q][:], in_=mprod[:, :, :])

        # ---- map broadcast: 3 sigma-group sub-DMAs on one queue, so the
        # first modulate op only waits on the first third ----
        def bcast(q, k, engine):
            md = maps_dram[q][0:1, 0:1]
            rep = reppool.tile([128, K2, QN], BF16, name="rep")
            flat = rep[:, :, :].rearrange("p a b -> p (a b)")
            for sg in range(3):
                engine.dma_start(
                    out=flat[:, 3 * sg * QN:3 * (sg + 1) * QN],
                    in_=bass.AP(tensor=md.tensor,
                                offset=md.offset + (k * K2 + 3 * sg) * QN,
                                ap=[[0, 128], [1, 3 * QN]]))
            return rep

        # ---- modulate: m[k][cc, t, :] = map_t (*) x window, 4 wide ops/k ----
        def mults(q, k, rep):
            ki, kj = divmod(k, 3)
            m = mpool.tile([128, 2, K2, QN], BF16, name="m", tag="m")
            r0 = q * QROWS + ki          # padded row of tap (rho=0, h=0)
            for cc in range(2):
                for sg in range(3):      # 3-tap (rho) column group per op
                    cs = kj + sg
                    xt, cb, rst = ((xs_bf, cs, WP) if cs % 2 == 0
                                   else (xs_bf1, cs - 1, WP - 2))
                    xin = _ap(xt[:, cc, r0, cb],
                              [[rst, 3], [rst, QROWS], [1, W]])
                    min_ = _ap(rep[:, 3 * sg, 0],
                               [[QN, 3], [W, QROWS], [1, W]])
                    mout = _ap(m[:, cc, 3 * sg, 0],
                               [[QN, 3], [W, QROWS], [1, W]])
                    nc.vector.tensor_tensor(mout, xin, min_, op=ALU.mult)
            return m

        # ---- main eighth bodies ----
        out_r = out_d[:].rearrange("one o h w -> (one o) h w")

        def id_group(m, k, j):
            """9 identity matmuls per cc accumulating taps into z psum."""
            zps = psum_z.tile([128, 2, HN], F32, tag="z", padded_shape=[128, 2, 512])
            for cc in range(2):
                for t in range(K2):
                    nc.tensor.matmul(zps[:, cc, :], lhsT=ident[:, :],
                                     rhs=m[:, cc, t, j * HN:(j + 1) * HN],
                                     start=(t == 0), stop=(t == K2 - 1))
            z_sb = zsbpool.tile([128, 2, HN], BF16, name="z_sb")
            nc.scalar.activation(z_sb[:, :, :], zps[:, :, :], AF.Copy)
            return z_sb

        out_ps = {}

        def w_group(q, k, j, z_sb):
            if k == 0:
                for oc in range(2):
                    out_ps[(j, oc)] = psum_out.tile([128, HN], F32, tag=f"o{oc}j{j}",
                                                    name=f"ops{oc}{j}")
            for cc in range(2):
                for oc in range(2):
                    nc.tensor.matmul(out_ps[(j, oc)][:, :],
                                     lhsT=wT[:, cc, k, oc * 128:(oc + 1) * 128],
                                     rhs=z_sb[:, cc, :],
                                     start=(k == 0 and cc == 0),
                                     stop=(k == K2 - 1 and cc == 1))
            if k == K2 - 1:
                e = 2 * q + j
                for oc in range(2):
                    osb = outpool.tile([128, HN], F32, name="osb")
                    nc.scalar.activation(osb[:, :], out_ps[(j, oc)][:, :], AF.Tanh,
                                         bias=bconv[:, oc:oc + 1])
                    nc.sync.dma_start(
                        out=out_r[oc * 128:(oc + 1) * 128,
                                  e * HROWS:(e + 1) * HROWS, :],
                        in_=osb[:, :])

        # ---- software-pipelined emission ----
        bcast_engines = [nc.gpsimd, nc.sync]
        bq = deque((q, k) for q in range(4) for k in range(K2))
        reps = {}
        n_bcast = 0

        def issue_bcast(ready_q):
            nonlocal n_bcast
            if bq and bq[0][0] <= ready_q:
                q, k = bq.popleft()
                reps[(q, k)] = bcast(q, k, bcast_engines[n_bcast % 2])
                n_bcast += 1

        def preamble(q):
            wyf = wyxpool.tile([K2, 3, QN], BF16, tag="wyf")
            wxf = wyxpool.tile([K2, 3, QN], BF16, tag="wxf")
            preamble_ab(q, 0, wyf, wxf)
            preamble_ab(q, 1, wyf, wxf)
            preamble_c(q, wyf, wxf)

        preamble(0)

        # ---- conv-weight transposes (needed only by the first W group) ----
        for cc in range(2):
            for g in range(5):
                kks = [2 * g, 2 * g + 1] if g < 4 else [8]
                tpw = psum_off.tile([128, 2 * len(kks), 128], BF16, tag="off",
                                    name="tpw")
                si = 0
                for kk in kks:
                    for oc in range(2):
                        srcw = wnat2[:, oc, :].rearrange("p (c k) -> p c k", k=K2)
                        nc.tensor.transpose(tpw[:, si, :],
                                            srcw[:, cc * 128:(cc + 1) * 128, kk],
                                            ident[:, :])
                        si += 1
                nc.vector.tensor_copy(
                    _ap(wT[:, cc, kks[0], 0], [[1, si * 128]]),
                    _ap(tpw[:, 0, 0], [[1, si * 128]]))
        for cc in range(2):
            nc.scalar.activation(wT[:, cc, :, :], wT[:, cc, :, :], AF.Copy,
                                 scale=s_se[:, cc, 0:1])

        ready = 0                     # highest q whose maps are stored
        pre_tiles = None
        for _ in range(5):
            issue_bcast(ready)
        ms = {}
        ms[(0, 0)] = mults(0, 0, reps.pop((0, 0)))
        pending_w = None              # (q, k, j, z_sb) awaiting emission

        for q in range(4):
            for k in range(K2):
                # lookahead: keep broadcast pipeline full, mults k+1
                issue_bcast(ready)
                issue_bcast(ready)
                nq, nk = (q, k + 1) if k + 1 < K2 else (q + 1, 0)
                if nq < 4:
                    ms[(nq, nk)] = mults(nq, nk, reps.pop((nq, nk)))
                m = ms.pop((q, k))
                z0 = id_group(m, k, 0)
                if pending_w is not None:
                    w_group(*pending_w)
                z1 = id_group(m, k, 1)
                w_group(q, k, 0, z0)
                pending_w = (q, k, 1, z1)
                if q < 3:
                    if k == 1:
                        pre_tiles = (wyxpool.tile([K2, 3, QN], BF16, tag="wyf",
                                                  name="wyf"),
                                     wyxpool.tile([K2, 3, QN], BF16, tag="wxf",
                                                  name="wxf"))
                        preamble_ab(q + 1, 0, *pre_tiles)
                    elif k == 2:
                        preamble_ab(q + 1, 1, *pre_tiles)
                    elif k == 3:
                        preamble_c(q + 1, *pre_tiles)
                        ready = q + 1
        w_group(*pending_w)
    nc.finalize()
    return nc


_NC = None


def _get_nc():
    global _NC
    if _NC is None:
        _NC = build()
    return _NC


def kernel(**inputs):
    global LAST_RESULT
    from concourse.bass_utils import run_bass_kernel_spmd

    nc = _get_nc()
    x = np.ascontiguousarray(inputs["x"], dtype=np.float32)
    shared = {k: np.ascontiguousarray(np.asarray(inputs[k]), dtype=np.float32)
              for k in ("w_off", "b_off", "w_conv", "b_conv", "fc1", "fc2")}
    in_maps = [{"x": x[i:i + 1], **shared} for i in range(B)]
    res = run_bass_kernel_spmd(nc, in_maps, core_ids=list(range(B)),
                               trace=bool(int(os.environ.get("KB_TRACE", "0"))))
    LAST_RESULT = res
    out = np.concatenate([res.results[i]["out"] for i in range(B)], axis=0)
    return out.astype(np.float32)


if __name__ == "__main__":
    nc = build()
    print("build OK")


# revision 26
# speedup vs baseline: 1.1010x; 1.0365x over previous
"""Trainium2 Bass kernel for AttentionDenseBlock (SE gate + offset conv + deform conv + tanh).

Strategy (per core, data-parallel over batch: 1 sample/core on 8 cores):
  - SE gate: spatial mean -> fc1 -> relu -> fc2 -> sigmoid; the per-channel
    scale s is FOLDED INTO THE WEIGHTS (wT, woffT scaled by s on ACT), so the
    input x is never rescaled and the DVE stays off the critical path.
  - Offset conv: 9 shifted bf16 matmuls accumulating in PSUM (s-scaled woffT).
  - Deform conv with |offset|<1 decomposes EXACTLY into a static 3x3-tap
    stencil per kernel position with data-dependent weights
    wy in {relu(-dy), 1-|dy|, relu(dy)} (x) wx likewise.
  - Main loop: for each kernel position k, the 9 modulated taps
    m_t = map_t (*) x_shift_t are accumulated in PSUM via IDENTITY matmuls
    (no weight reloads), the f32 sum z_k is copied to SBUF (ACT) and hit with
    ONE W_k matmul event: ~0.65us/tap on PE vs 1.31us/tap (full W matmul) or
    0.92us/tap (DVE adds). DVE does only the 9 modulate multiplies per k,
    merged into 4 wide multi-tap ops (4 free dims).
  - Spatial processed in 8 "eighths" of 7 rows (HN=392 = one PSUM bank f32):
    out-psum (4 banks) + z-psum (2) + offset-psum (2) = all 8 banks.
  - PSUM->SBUF z copies hidden under the next W matmul group via the
    interleave: id(k,j0) W(k-1,j1) id(k,j1) W(k,j0).
  - Tap maps are built per quarter on 9-partition tiles (ACT relus + one
    9-tap DVE product op), stored to DRAM in one DMA, and broadcast to 128
    partitions with one contiguous-read DMA per (q,k) on 3 rotating queues.
"""

import os
import sys
from collections import deque
from contextlib import ExitStack

import numpy as np

sys.path.insert(0, "/opt/trn_rl_repo")

import concourse.bass as bass
import concourse.bacc as bacc
import concourse.mybir as mybir
import concourse.tile as tile
from concourse.masks import make_identity

B, C, O, H, W = 8, 256, 256, 56, 56
KH = KW = 3
K2 = 9
HP, WP = H + 4, W + 4  # zero-padded by 2 for the 5x5 shift range
HW = H * W
QROWS = 14            # rows per quarter (map granularity)
QN = HW // 4          # 784 spatial positions per quarter
HROWS = 7             # rows per eighth (psum granularity)
HN = QN // 2          # 392 = one PSUM bank of f32
RED = 16              # SE reduction dim

F32 = mybir.dt.float32
BF16 = mybir.dt.bfloat16
AF = mybir.ActivationFunctionType
ALU = mybir.AluOpType

LAST_RESULT = None


def _ap(base, extra_dims):
    """AP on `base`'s tensor starting at base's offset with explicit free dims."""
    return bass.AP(tensor=base.tensor, offset=base.offset,
                   ap=[list(base.ap[0])] + [list(d) for d in extra_dims])


def build():
    nc = bacc.Bacc()
    x_d = nc.dram_tensor("x", (1, C, H, W), F32, kind="ExternalInput")
    woff_d = nc.dram_tensor("w_off", (2 * K2, C, KH, KW), F32, kind="ExternalInput")
    boff_d = nc.dram_tensor("b_off", (2 * K2,), F32, kind="ExternalInput")
    wconv_d = nc.dram_tensor("w_conv", (O, C, KH, KW), F32, kind="ExternalInput")
    bconv_d = nc.dram_tensor("b_conv", (O,), F32, kind="ExternalInput")
    fc1_d = nc.dram_tensor("fc1", (RED, C), F32, kind="ExternalInput")
    fc2_d = nc.dram_tensor("fc2", (C, RED), F32, kind="ExternalInput")
    out_d = nc.dram_tensor("out", (1, O, H, W), F32, kind="ExternalOutput")

    with tile.TileContext(nc) as tc, ExitStack() as ctx:
        singles = ctx.enter_context(tc.tile_pool(name="singles", bufs=1))
        wyxpool = ctx.enter_context(tc.tile_pool(name="wyxpool", bufs=1))
        mpool = ctx.enter_context(tc.tile_pool(name="mpool", bufs=2))
        reppool = ctx.enter_context(tc.tile_pool(name="reppool", bufs=6))
        zsbpool = ctx.enter_context(tc.tile_pool(name="zsbpool", bufs=4))
        outpool = ctx.enter_context(tc.tile_pool(name="outpool", bufs=2))
        dpool = ctx.enter_context(tc.tile_pool(name="dpool", bufs=1, space="DRAM"))
        psum_z = ctx.enter_context(tc.tile_pool(name="psum_z", bufs=1, space="PSUM"))
        psum_out = ctx.enter_context(tc.tile_pool(name="psum_out", bufs=1, space="PSUM"))
        psum_off = ctx.enter_context(tc.tile_pool(name="psum_off", bufs=2, space="PSUM"))

        # ---- static tiles ----
        xs_bf = singles.tile([128, 2, HP, WP], BF16)     # raw x, padded, even phase
        xs_bf1 = singles.tile([128, 2, HP, WP - 2], BF16)  # odd phase: col c = col c+1
        wT = singles.tile([128, 2, K2, O], BF16)         # [c, cc, k, o]; s-scaled
        # woffT free dim: [0:9]=dy weights, [32:41]=dx weights (aligned blocks)
        woffT = singles.tile([128, 2, K2, 48], BF16)
        fc1T = singles.tile([128, 2, RED], F32)
        fc2T = singles.tile([128, C], F32)
        bconv = singles.tile([128, 2], F32)
        boff = singles.tile([64, 1], F32)
        y_se = singles.tile([128, 2, 1], F32)
        h_se = singles.tile([128, 1], F32)
        s_se = singles.tile([128, 2, 1], F32)
        boffn = singles.tile([64, 1], F32)
        ident = singles.tile([128, 128], BF16)
        wnat2 = mpool.tile([128, 2, C * K2], BF16, name="wnat2", tag="wn",
                           bufs=1)
        woff_nat2 = mpool.tile([2 * K2, C * K2], BF16, name="woff_nat2", tag="m",
                               padded_shape=[2 * K2, 2 * K2 * QN])
        # k-major tap maps per quarter: row k*9 + (3r+s)
        maps_dram = [dpool.tile([K2 * K2, QN], BF16, name=f"maps{q}") for q in range(4)]

        make_identity(nc, ident[:, :])

        # ---- weight DMA first (cast to bf16 on SWDGE) so PE transposes start
        # early, then the big x input DMA ----
        wc_r = wconv_d[:].rearrange("o c kh kw -> o (c kh kw)")
        nc.gpsimd.dma_start(out=woff_nat2[:, :],
                            in_=woff_d[:].rearrange("o c kh kw -> o (c kh kw)"))
        for oc in range(2):
            nc.gpsimd.dma_start(out=wnat2[:, oc, :], in_=wc_r[oc * 128:(oc + 1) * 128, :])
        nc.vector.memset(xs_bf[:, :, :, :], 0.0)
        nc.vector.memset(xs_bf1[:, :, :, :], 0.0)
        x_r = x_d[:].rearrange("one c h w -> (one c) h w")
        HH = H // 2
        for hh in range(2):
            nc.gpsimd.dma_start(
                out=xs_bf[:, 0, 2 + hh * HH:2 + (hh + 1) * HH, 2:2 + W],
                in_=x_r[0:128, hh * HH:(hh + 1) * HH, :])
        xstage = reppool.tile([128, H, W], F32, name="xstage", tag="rep")
        for hh in range(2):
            nc.sync.dma_start(out=xstage[:, hh * HH:(hh + 1) * HH, :],
                              in_=x_r[128:256, hh * HH:(hh + 1) * HH, :])
            nc.scalar.activation(
                xs_bf[:, 1, 2 + hh * HH:2 + (hh + 1) * HH, 2:2 + W],
                xstage[:, hh * HH:(hh + 1) * HH, :], AF.Copy)
        fc1_r = fc1_d[:].rearrange("m c -> c m")
        for cc in range(2):
            nc.sync.dma_start(out=fc1T[:, cc, :], in_=fc1_r[cc * 128:(cc + 1) * 128, :])
        nc.vector.memset(fc2T[:, :], 0.0)
        nc.sync.dma_start(out=fc2T[0:RED, :], in_=fc2_d[:].rearrange("c m -> m c"))
        nc.sync.dma_start(out=bconv[:, :],
                          in_=bconv_d[:].rearrange("(a c) -> c a", a=2))
        # b_off loaded de-interleaved: dy biases -> rows 0:9, dx -> rows 32:41
        nc.vector.memset(boff[:, :], 0.0)
        boff_src = boff_d[:]
        nc.sync.dma_start(out=boff[0:K2, 0:1],
                          in_=bass.AP(tensor=boff_src.tensor, offset=boff_src.offset,
                                      ap=[[2, K2], [0, 1]]))
        nc.sync.dma_start(out=boff[32:32 + K2, 0:1],
                          in_=bass.AP(tensor=boff_src.tensor,
                                      offset=boff_src.offset + 1,
                                      ap=[[2, K2], [0, 1]]))

        nc.vector.memset(woffT[:, :, :, :], 0.0)

        # ---- offset-weight transposes (gate the offset conv), batched ----
        for cc in range(2):
            srco = woff_nat2[:, :].rearrange("p (c k) -> p c k", k=K2)
            for g in range(3):
                tpo = psum_off.tile([128, 3, 128], BF16, tag="off", name="tpo")
                for i in range(3):
                    kk = 3 * g + i
                    nc.tensor.transpose(tpo[:, i, 0:2 * K2],
                                        srco[:, cc * 128:(cc + 1) * 128, kk],
                                        ident[0:2 * K2, 0:2 * K2])
                # de-interleave offset channels: dy -> cols 0:9, dx -> 32:41
                nc.vector.tensor_copy(
                    _ap(woffT[:, cc, 3 * g, 0], [[48, 3], [1, K2]]),
                    _ap(tpo[:, 0, 0], [[128, 3], [2, K2]]))
                nc.vector.tensor_copy(
                    _ap(woffT[:, cc, 3 * g, 32], [[48, 3], [1, K2]]),
                    _ap(tpo[:, 0, 1], [[128, 3], [2, K2]]))

        # odd column phase of raw x (cols 0..57 used; col 58/59 never read)
        for cc in range(2):
            nc.scalar.activation(xs_bf1[:, cc, :, 0:WP - 2], xs_bf[:, cc, :, 1:WP - 1],
                                 AF.Copy)

        # ---- SE gate -> s; fold s into wTs / woffTs on ACT ----
        y_part = singles.tile([128, 2, 2], F32)
        for cc in range(2):
            for hh in range(2):
                nc.vector.tensor_reduce(
                    out=y_part[:, cc, hh:hh + 1],
                    in_=xs_bf[:, cc, 2 + hh * (H // 2):2 + (hh + 1) * (H // 2), 2:2 + W],
                    axis=mybir.AxisListType.XY, op=ALU.add)
        for cc in range(2):
            nc.vector.tensor_reduce(out=y_se[:, cc, 0:1], in_=y_part[:, cc, :],
                                    axis=mybir.AxisListType.X, op=ALU.add)
        nc.vector.tensor_scalar_mul(y_se[:, :, 0:1], y_se[:, :, 0:1], 1.0 / HW)
        h_ps = psum_z.tile([128, RED], F32, tag="z")
        for cc in range(2):
            nc.tensor.matmul(h_ps[0:RED, 0:1], lhsT=fc1T[:, cc, :], rhs=y_se[:, cc, 0:1],
                             start=(cc == 0), stop=(cc == 1))
        nc.vector.memset(h_se[:, :], 0.0)
        nc.vector.tensor_relu(h_se[0:RED, 0:1], h_ps[0:RED, 0:1])
        for cc in range(2):
            s_ps = psum_z.tile([128, RED], F32, tag="z")
            nc.tensor.matmul(s_ps[:, 0:1], lhsT=fc2T[:, cc * 128:(cc + 1) * 128],
                             rhs=h_se[:, 0:1], start=True, stop=True)
            nc.scalar.activation(s_se[:, cc, 0:1], s_ps[:, 0:1], AF.Sigmoid)
        for cc in range(2):
            nc.scalar.activation(woffT[:, cc, :, 0:48], woffT[:, cc, :, 0:48],
                                 AF.Copy, scale=s_se[:, cc, 0:1])

        nc.scalar.activation(boffn[:, 0:1], boff[:, 0:1], AF.Copy, scale=-1.0)

        # ---- per-quarter preamble, staged: offset conv -> maps -> DRAM ----
        def preamble_ab(q, nn, wyf, wxf):
            if True:
                off_ps = psum_off.tile([64, HN], F32, tag="off")
                for kk in range(K2):
                    ki, kj = divmod(kk, 3)
                    dh, dw = ki - 1, kj - 1
                    for cc in range(2):
                        r0 = 2 + dh + q * QROWS + nn * HROWS
                        rhs = xs_bf[:, cc, r0:r0 + HROWS, 2 + dw:2 + dw + W]
                        nc.tensor.matmul(off_ps[0:48, :],
                                         lhsT=woffT[:, cc, kk, 0:48], rhs=rhs,
                                         start=(kk == 0 and cc == 0),
                                         stop=(kk == K2 - 1 and cc == 1))
                # offset = psum + b_off, fused into relu(+-offset) tap weights
                nsl = slice(nn * HN, (nn + 1) * HN)
                nc.scalar.activation(wyf[:, 0, nsl], off_ps[0:K2, :], AF.Relu,
                                     scale=-1.0, bias=boffn[0:K2, 0:1])
                nc.scalar.activation(wyf[:, 2, nsl], off_ps[0:K2, :], AF.Relu,
                                     scale=1.0, bias=boff[0:K2, 0:1])
                nc.scalar.activation(wxf[:, 0, nsl], off_ps[32:32 + K2, :], AF.Relu,
                                     scale=-1.0, bias=boffn[32:32 + K2, 0:1])
                nc.scalar.activation(wxf[:, 2, nsl], off_ps[32:32 + K2, :], AF.Relu,
                                     scale=1.0, bias=boff[32:32 + K2, 0:1])
        def preamble_c(q, wyf, wxf):
            # wy1 = 1 - |dy| = 1 - (relu(dy) + relu(-dy)); same for wx1
            nc.vector.tensor_add(wyf[:, 1, :], wyf[:, 0, :], wyf[:, 2, :])
            nc.scalar.activation(wyf[:, 1, :], wyf[:, 1, :], AF.Copy, scale=-1.0,
                                 bias=1.0)
            nc.vector.tensor_add(wxf[:, 1, :], wxf[:, 0, :], wxf[:, 2, :])
            nc.scalar.activation(wxf[:, 1, :], wxf[:, 1, :], AF.Copy, scale=-1.0,
                                 bias=1.0)
            # all 9 tap products in ONE DVE op (sigma-major rows t = 3s + r),
            # one DMA store k-major
            mprod = mpool.tile([K2, K2, QN], BF16, name="mprod", tag="m")
            win = _ap(wyf[0:K2, 0, 0], [[0, 3], [QN, 3], [1, QN]])
            xin = _ap(wxf[0:K2, 0, 0], [[QN, 3], [0, 3], [1, QN]])
            mout = _ap(mprod[0:K2, 0, 0], [[3 * QN, 3], [QN, 3], [1, QN]])
            nc.vector.tensor_tensor(mout, win, xin, op=ALU.mult)
            nc.scalar.dma_start(out=maps_dram[q][:], in_=mprod[:, :, :])

        # ---- map broadcast: 3 sigma-group sub-DMAs on one queue, so the
        # first modulate op only waits on the first third ----
        def bcast(q, k, engine):
            md = maps_dram[q][0:1, 0:1]
            rep = reppool.tile([128, K2, QN], BF16, name="rep")
            flat = rep[:, :, :].rearrange("p a b -> p (a b)")
            for sg in range(3):
                engine.dma_start(
                    out=flat[:, 3 * sg * QN:3 * (sg + 1) * QN],
                    in_=bass.AP(tensor=md.tensor,
                                offset=md.offset + (k * K2 + 3 * sg) * QN,
                                ap=[[0, 128], [1, 3 * QN]]))
            return rep

        # ---- modulate: m[k][cc, t, :] = map_t (*) x window, 4 wide ops/k ----
        def mults(q, k, rep):
            ki, kj = divmod(k, 3)
            m = mpool.tile([128, 2, K2, QN], BF16, name="m", tag="m")
            r0 = q * QROWS + ki          # padded row of tap (rho=0, h=0)
            for cc in range(2):
                for sg in range(3):      # 3-tap (rho) column group per op
                    cs = kj + sg
                    xt, cb, rst = ((xs_bf, cs, WP) if cs % 2 == 0
                                   else (xs_bf1, cs - 1, WP - 2))
                    xin = _ap(xt[:, cc, r0, cb],
                              [[rst, 3], [rst, QROWS], [1, W]])
                    min_ = _ap(rep[:, 3 * sg, 0],
                               [[QN, 3], [W, QROWS], [1, W]])
                    mout = _ap(m[:, cc, 3 * sg, 0],
                               [[QN, 3], [W, QROWS], [1, W]])
                    nc.vector.tensor_tensor(mout, xin, min_, op=ALU.mult)
            return m

        # ---- main eighth bodies ----
        out_r = out_d[:].rearrange("one o h w -> (one o) h w")

        def id_group(m, k, j):
            """9 identity matmuls per cc accumulating taps into z psum."""
            zps = psum_z.tile([128, 2, HN], F32, tag="z", padded_shape=[128, 2, 512])
            for cc in range(2):
                for t in range(K2):
                    nc.tensor.matmul(zps[:, cc, :], lhsT=ident[:, :],
                                     rhs=m[:, cc, t, j * HN:(j + 1) * HN],
                                     start=(t == 0), stop=(t == K2 - 1))
            z_sb = zsbpool.tile([128, 2, HN], BF16, name="z_sb")
            nc.scalar.activation(z_sb[:, :, :], zps[:, :, :], AF.Copy)
            return z_sb

        out_ps = {}

        def w_group(q, k, j, z_sb):
            if k == 0:
                for oc in range(2):
                    out_ps[(j, oc)] = psum_out.tile([128, HN], F32, tag=f"o{oc}j{j}",
                                                    name=f"ops{oc}{j}")
            for cc in range(2):
                for oc in range(2):
                    nc.tensor.matmul(out_ps[(j, oc)][:, :],
                                     lhsT=wT[:, cc, k, oc * 128:(oc + 1) * 128],
                                     rhs=z_sb[:, cc, :],
                                     start=(k == 0 and cc == 0),
                                     stop=(k == K2 - 1 and cc == 1))
            if k == K2 - 1:
                e = 2 * q + j
                for oc in range(2):
                    osb = outpool.tile([128, HN], F32, name="osb")
                    nc.scalar.activation(osb[:, :], out_ps[(j, oc)][:, :], AF.Tanh,
                                         bias=bconv[:, oc:oc + 1])
                    nc.sync.dma_start(
                        out=out_r[oc * 128:(oc + 1) * 128,
                                  e * HROWS:(e + 1) * HROWS, :],
                        in_=osb[:, :])

        # ---- software-pipelined emission ----
        bcast_engines = [nc.gpsimd, nc.sync]
        bq = deque((q, k) for q in range(4) for k in range(K2))
        reps = {}
        n_bcast = 0

        def issue_bcast(ready_q):
            nonlocal n_bcast
            if bq and bq[0][0] <= ready_q:
                q, k = bq.popleft()
                reps[(q, k)] = bcast(q, k, bcast_engines[n_bcast % 2])
                n_bcast += 1

        def preamble(q):
            wyf = wyxpool.tile([K2, 3, QN], BF16, tag="wyf")
            wxf = wyxpool.tile([K2, 3, QN], BF16, tag="wxf")
            preamble_ab(q, 0, wyf, wxf)
            preamble_ab(q, 1, wyf, wxf)
            preamble_c(q, wyf, wxf)

        preamble(0)

        # ---- conv-weight transposes (needed only by the first W group) ----
        for cc in range(2):
            for g in range(5):
                kks = [2 * g, 2 * g + 1] if g < 4 else [8]
                tpw = psum_off.tile([128, 2 * len(kks), 128], BF16, tag="off",
                                    name="tpw")
                si = 0
                for kk in kks:
                    for oc in range(2):
                        srcw = wnat2[:, oc, :].rearrange("p (c k) -> p c k", k=K2)
                        nc.tensor.transpose(tpw[:, si, :],
                                            srcw[:, cc * 128:(cc + 1) * 128, kk],
                                            ident[:, :])
                        si += 1
                nc.vector.tensor_copy(
                    _ap(wT[:, cc, kks[0], 0], [[1, si * 128]]),
                    _ap(tpw[:, 0, 0], [[1, si * 128]]))
        for cc in range(2):
            nc.scalar.activation(wT[:, cc, :, :], wT[:, cc, :, :], AF.Copy,
                                 scale=s_se[:, cc, 0:1])

        ready = 0                     # highest q whose maps are stored
        pre_tiles = None
        for _ in range(5):
            issue_bcast(ready)
        ms = {}
        ms[(0, 0)] = mults(0, 0, reps.pop((0, 0)))
        pending_w = None              # (q, k, j, z_sb) awaiting emission

        for q in range(4):
            for k in range(K2):
                # lookahead: keep broadcast pipeline full, mults k+1
                issue_bcast(ready)
                issue_bcast(ready)
                issue_bcast(ready)
                nq, nk = (q, k + 1) if k + 1 < K2 else (q + 1, 0)
                if nq < 4:
                    ms[(nq, nk)] = mults(nq, nk, reps.pop((nq, nk)))
                m = ms.pop((q, k))
                z0 = id_group(m, k, 0)
                if pending_w is not None:
                    w_group(*pending_w)
                z1 = id_group(m, k, 1)
                w_group(q, k, 0, z0)
                pending_w = (q, k, 1, z1)
                if q < 3:
                    if k == 1:
                        pre_tiles = (wyxpool.tile([K2, 3, QN], BF16, tag="wyf",
                                                  name="wyf"),
                                     wyxpool.tile([K2, 3, QN], BF16, tag="wxf",
                                                  name="wxf"))
                        preamble_ab(q + 1, 0, *pre_tiles)
                    elif k == 2:
                        preamble_ab(q + 1, 1, *pre_tiles)
                    elif k == 3:
                        preamble_c(q + 1, *pre_tiles)
                        ready = q + 1
        w_group(*pending_w)
    nc.finalize()
    return nc


_NC = None


def _get_nc():
    global _NC
    if _NC is None:
        _NC = build()
    return _NC


def kernel(**inputs):
    global LAST_RESULT
    from concourse.bass_utils import run_bass_kernel_spmd

    nc = _get_nc()
    x = np.ascontiguousarray(inputs["x"], dtype=np.float32)
    shared = {k: np.ascontiguousarray(np.asarray(inputs[k]), dtype=np.float32)
              for k in ("w_off", "b_off", "w_conv", "b_conv", "fc1", "fc2")}
    in_maps = [{"x": x[i:i + 1], **shared} for i in range(B)]
    res = run_bass_kernel_spmd(nc, in_maps, core_ids=list(range(B)),
                               trace=bool(int(os.environ.get("KB_TRACE", "0"))))
    LAST_RESULT = res
    out = np.concatenate([res.results[i]["out"] for i in range(B)], axis=0)
    return out.astype(np.float32)


if __name__ == "__main__":
    nc = build()
    print("build OK")
